# revision 16
# baseline (speedup 1.0000x reference)
"""Multi-head self-attention Trainium2 kernel (B=2, S=2048, D=1024, H=32, d=32).

Sharding: 8 cores = (batch b in {0,1}) x (query quarter qc in {0..3}).
Each core holds x[b] fully (for K/V over all 2048 keys) and computes the
attention + output projection for its 512 queries. Host concatenates.

Per-core pipeline (bf16 operands, fp32 PSUM accumulation):
  xT [D,S] cast to bf16 on load; block-diagonal 4-head K/Q/V projections;
  scores transposed [keys, q] via PE row-tiling (tile_position (32j, 0), one
  PSUM bank per concurrent row tile); exp on ACT with fused 1/sqrt(d) scale,
  no max subtraction (|s| <= ~12 for randn inputs); attn@v in [e, q]
  orientation: out[(e|sum), q] accumulates per head at column strip {0,64} x
  bank {0,1}, with a ones-column in v producing softmax denominators in the
  strip+32 row; normalize = reciprocal_approx on the sums row, K=1
  ones-matmul broadcast of the reciprocal across the strip, then one DVE
  multiply straight into the concatT chunk (so no transposes at all); output
  projection contracts 16 half-dense chunks against a host-reordered,
  zero-padded wo.
PSUM discipline: concurrent row/col-tiled MMs target distinct banks or
documented col strips; shared-bank accumulators are opened by a zero matmul
(start=True clears has_written bank-wide) and accumulated with start=False;
opener zeros also make the dead strips read as 0 so the normalize multiply
writes exact zeros where wo is zero-padded.
"""
import numpy as np
import ml_dtypes

import concourse.bacc as bacc
import concourse.mybir as mybir
import concourse.tile as tile
from concourse import bass_utils

f32 = mybir.dt.float32
bf16 = mybir.dt.bfloat16
AF = mybir.ActivationFunctionType

B, S, D, H, dh = 2, 2048, 1024, 32, 32
NCORES = 8
QCH = S // (NCORES // B)      # 512 queries per core
NHG = D // 128                # 8 four-head groups
NKC = S // 128                # 16 key chunks
NQS = QCH // 128              # 4 query sub-chunks
SCALE = 1.0 / float(np.sqrt(dh))


def build_module(loop_iters: int = 0, stage: int = 6):
    nc = bacc.Bacc("TRN2", target_bir_lowering=False, debug=False)
    xt_d = nc.dram_tensor("xt", [D, S], f32, kind="ExternalInput")
    xq_d = nc.dram_tensor("xq", [D, QCH], f32, kind="ExternalInput")
    wqbd_d = nc.dram_tensor("wqbd", [128, 128], bf16, kind="ExternalInput")
    wkbd_d = nc.dram_tensor("wkbd", [128, 128], bf16, kind="ExternalInput")
    wvbd_d = nc.dram_tensor("wvbd", [128, 128], bf16, kind="ExternalInput")
    wop_d = nc.dram_tensor("wop", [16 * 128, D], bf16, kind="ExternalInput")
    out_d = nc.dram_tensor("out", [QCH, D], f32, kind="ExternalOutput")

    with tile.TileContext(nc) as tc:
        with (
            tc.tile_pool(name="pers", bufs=1) as pers,
            tc.tile_pool(name="sbk", bufs=2) as sbk,
            tc.tile_pool(name="sbe", bufs=4) as sbe,
            tc.tile_pool(name="sbv", bufs=18) as sbv,
            tc.tile_pool(name="sbx", bufs=4) as sbx,
            tc.tile_pool(name="psS", bufs=2, space="PSUM") as psS,
            tc.tile_pool(name="psO", bufs=1, space="PSUM") as psO,
            tc.tile_pool(name="psA", bufs=2, space="PSUM") as psA,
        ):
            def body(_iv=None):
                XT = []
                for t in range(NHG):
                    xtt = pers.tile([128, S], bf16, name=f"XT{t}", tag=f"XT{t}")
                    nc.gpsimd.dma_start(xtt[:, :], xt_d[128 * t:128 * (t + 1), :])
                    XT.append(xtt)
                XQ = []
                for t in range(NHG):
                    xqt = pers.tile([128, QCH], bf16, name=f"XQ{t}", tag=f"XQ{t}")
                    nc.gpsimd.dma_start(xqt[:, :], xq_d[128 * t:128 * (t + 1), :])
                    XQ.append(xqt)
                WOP = []
                for t in range(16):
                    wot = pers.tile([128, D], bf16, name=f"WOP{t}", tag=f"WOP{t}")
                    nc.sync.dma_start(wot[:, :], wop_d[128 * t:128 * (t + 1), :])
                    WOP.append(wot)
                wqbd = pers.tile([128, 128], bf16, name="wqbd", tag="wqbd")
                nc.sync.dma_start(wqbd[:, :], wqbd_d[:, :])
                wkbd = pers.tile([128, 128], bf16, name="wkbd", tag="wkbd")
                nc.sync.dma_start(wkbd[:, :], wkbd_d[:, :])
                wvbd = pers.tile([128, 128], bf16, name="wvbd", tag="wvbd")
                nc.sync.dma_start(wvbd[:, :], wvbd_d[:, :])
                zrow = pers.tile([1, 640], bf16, name="zrow", tag="zrow")
                nc.vector.memset(zrow[:, :], 0.0)
                onesf = pers.tile([128, 64], f32, name="onesf", tag="onesf")
                nc.vector.memset(onesf[:, :], 1.0)

                if stage <= 1:
                    sink = pers.tile([128, 128], bf16, name="sink", tag="sink")
                    for t in range(NHG):
                        nc.vector.tensor_copy(sink[:, :], XT[t][:, 0:128])
                        nc.vector.tensor_copy(sink[:, :], XQ[t][:, 0:128])
                    for t in range(16):
                        nc.vector.tensor_copy(sink[:, :], WOP[t][:, 0:128])
                    nc.vector.tensor_copy(sink[:, :], wqbd[:, :])
                    nc.vector.tensor_copy(sink[:, :], wkbd[:, :])
                    nc.vector.tensor_copy(sink[:, :], wvbd[:, :])
                    nc.vector.tensor_copy(sink[0:1, 0:128], zrow[:, 0:128])
                    nc.vector.tensor_copy(sink[:, 0:64].bitcast(f32),
                                          onesf[:, :])
                    sinkf = pers.tile([128, 128], f32, name="sinkf", tag="sinkf")
                    nc.vector.tensor_copy(sinkf[:, :], sink[:, :])
                    nc.sync.dma_start(out_d[0:128, 0:128], sinkf[:, :])
                    return

                CT = []
                if stage >= 6:
                    for c in range(16):
                        ctt = pers.tile([128, QCH], bf16, name=f"CT{c}",
                                        tag=f"CT{c}")
                        CT.append(ctt)
                OUTSB = []
                if stage >= 6:
                    for qs in range(NQS):
                        ot = pers.tile([128, D], f32, name=f"OUTSB{qs}",
                                       tag=f"OUTSB{qs}")
                        if stage < 7:
                            nc.vector.memset(ot[:, :], 0.0)
                        OUTSB.append(ot)

                pending_tails = []
                pending_projs = []
                for hg in range(NHG):
                    # ---- K/Q projections (block-diagonal, 4 heads at once)
                    kt4 = sbk.tile([128, S], bf16, name=f"kt4_{hg}", tag="kt")
                    for sc in range(S // 512):
                        pk = psA.tile([128, 512], f32, name=f"pk{hg}_{sc}",
                                      tag="aux")
                        nc.tensor.matmul(pk[:, :], wkbd[:, :],
                                         XT[hg][:, 512 * sc:512 * (sc + 1)],
                                         start=True, stop=True)
                        nc.vector.tensor_copy(kt4[:, 512 * sc:512 * (sc + 1)],
                                              pk[:, :])
                    qt4 = sbk.tile([128, QCH], bf16, name=f"qt4_{hg}", tag="qt")
                    pq = psA.tile([128, 512], f32, name=f"pq{hg}", tag="aux")
                    nc.tensor.matmul(pq[:, :], wqbd[:, :], XQ[hg][:, :],
                                     start=True, stop=True)
                    nc.vector.tensor_copy(qt4[:, :], pq[:, :])

                    # ---- V tiles burst (packed psum: 4 key-chunks per bank)
                    vts = []
                    for kq in range(4):
                        pv = psA.tile([128, 512], f32, name=f"pv{hg}_{kq}",
                                      tag="aux")
                        for u in range(4):
                            kc = 4 * kq + u
                            nc.tensor.matmul(
                                pv[:, 128 * u:128 * (u + 1)],
                                XT[hg][:, 128 * kc:128 * (kc + 1)],
                                wvbd[:, :],
                                start=(u == 0), stop=(u == 3),
                                skip_group_check=True)
                        for u in range(4):
                            kc = 4 * kq + u
                            vt = sbv.tile([128, 132], bf16,
                                          name=f"vt{hg}_{kc}", tag="v")
                            nc.vector.tensor_copy(
                                vt[:, :].rearrange("p (h e) -> p h e",
                                                   h=4)[:, :, 0:32],
                                pv[:, 128 * u:128 * (u + 1)].rearrange(
                                    "p (h e) -> p h e", h=4))
                            nc.vector.memset(
                                vt[:, :].rearrange("p (h e) -> p h e",
                                                   h=4)[:, :, 32:33], 1.0)
                            vts.append(vt)
                    if stage <= 2:
                        continue

                    # ---- attn@v accumulator: head j -> bank j//2 (via the
                    # free-dim 512-offset), col strip 64*(j%2); openers zero
                    # both banks so start=False accumulation + dead strips work
                    po = psO.tile([128, 1024], f32, name=f"po{hg}", tag="o")
                    if stage >= 5:
                        for bank in range(2):
                            nc.tensor.matmul(po[:, 512 * bank:512 * (bank + 1)],
                                             zrow[:, 0:128], zrow[:, 128:640],
                                             start=True, stop=True,
                                             skip_group_check=True)

                    def attnv(kc, ets_kc):
                        # head j: lhsT = v_aug slice (33 cols), rhs = et half,
                        # out [33, 512] at (bank j//2, strip 64*(j%2));
                        # order 0,2,1,3 alternates banks
                        for j in (0, 2, 1, 3):
                            nc.tensor.matmul(
                                po[:, 512 * (j // 2):512 * (j // 2) + 512][
                                    64 * (j % 2):64 * (j % 2) + 33, :],
                                vts[kc][:, 33 * j:33 * (j + 1)],
                                ets_kc[j // 2][:, 512 * (j % 2):
                                               512 * (j % 2) + 512],
                                start=False, stop=(kc == NKC - 1),
                                skip_group_check=True)

                    prev = None
                    for kc in range(NKC):
                        sss = []
                        for pr in range(2):
                            ss = psS.tile([128, 1024], f32,
                                          name=f"ss{hg}_{kc}_{pr}", tag="s")
                            for jj in range(2):
                                j = 2 * pr + jj
                                nc.tensor.matmul(
                                    ss[:, 512 * jj:512 * (jj + 1)],
                                    kt4[32 * j:32 * (j + 1),
                                        128 * kc:128 * (kc + 1)],
                                    qt4[32 * j:32 * (j + 1), :],
                                    start=True, stop=True,
                                    tile_position=(32 * j, 0))
                            sss.append(ss)
                        ets = []
                        for pr in range(2):
                            et = sbe.tile([128, 1024], bf16,
                                          name=f"et{hg}_{kc}_{pr}", tag="e")
                            if stage <= 3:
                                nc.vector.tensor_copy(et[:, :], sss[pr][:, :])
                            else:
                                nc.scalar.activation(et[:, :], sss[pr][:, :],
                                                     AF.Exp, scale=SCALE)
                            ets.append(et)
                        if stage >= 5 and prev is not None:
                            attnv(prev[0], prev[1])
                        prev = (kc, ets)
                    if stage >= 5:
                        attnv(prev[0], prev[1])
                    if stage <= 5:
                        continue

                    # ---- evacuate po once (releases the accumulator
                    # banks for the next head group); normalize + incremental
                    # out-projection are DEFERRED one head group so their sem
                    # chains never stall the next group's score/attn stream
                    pof = sbx.tile([128, 1024], f32, name=f"pof{hg}", tag="pof",
                                   bufs=3)
                    nc.vector.tensor_copy(pof[:, :], po[:, :])

                    def make_tail(hg, pof):
                        def tail():
                            for bank in range(2):
                                c = 2 * hg + bank
                                pobf = pof[:, 512 * bank:512 * (bank + 1)]
                                pb = psA.tile([128, 512], f32,
                                              name=f"pb{hg}_{bank}", tag="aux")
                                for sj in range(2):
                                    strip = 64 * sj
                                    nc.tensor.matmul(
                                        pb[strip:strip + 64, :],
                                        onesf[strip + 32:strip + 33, :],
                                        pobf[strip + 32:strip + 33, :],
                                        start=True, stop=True,
                                        tile_position=(strip + 32, strip))
                                bc = sbx.tile([128, 512], f32,
                                              name=f"bc{hg}_{bank}", tag="bc")
                                nc.vector.tensor_copy(bc[:, :], pb[:, :])
                                rbc = sbx.tile([128, 512], f32,
                                               name=f"rbc{hg}_{bank}",
                                               tag="rbc")
                                nc.vector.reciprocal_approx_fast(rbc[:, :],
                                                                 bc[:, :])
                                for sj in range(2):
                                    strip = 64 * sj
                                    nc.vector.tensor_mul(
                                        CT[c][strip:strip + 64, :],
                                        pobf[strip:strip + 64, :],
                                        rbc[strip:strip + 64, :])
                        return tail

                    def make_proj(hg):
                        def proj():
                            if stage >= 7:
                                return
                            # incremental out-projection for chunks 2hg, 2hg+1
                            for qs in range(NQS):
                                for og in range(2):
                                    pe_ = psA.tile([128, 512], f32,
                                                   name=f"pe{hg}_{qs}_{og}",
                                                   tag="aux")
                                    for cc in range(2):
                                        c = 2 * hg + cc
                                        nc.tensor.matmul(
                                            pe_[:, :],
                                            CT[c][:, 128 * qs:128 * (qs + 1)],
                                            WOP[c][:, 512 * og:512 * (og + 1)],
                                            start=(cc == 0), stop=(cc == 1))
                                    nc.vector.tensor_add(
                                        OUTSB[qs][:, 512 * og:512 * (og + 1)],
                                        OUTSB[qs][:, 512 * og:512 * (og + 1)],
                                        pe_[:, :])
                        return proj

                    pending_tails.append(make_tail(hg, pof))
                    if len(pending_tails) > 1:
                        pending_tails.pop(0)()
                    pending_projs.append(make_proj(hg))
                    if len(pending_projs) > 2:
                        pending_projs.pop(0)()

                if stage <= 5:
                    return
                for t_ in pending_tails:
                    t_()
                for p_ in pending_projs:
                    p_()
                if stage >= 7:
                    return
                for qs in range(NQS):
                    nc.sync.dma_start(out_d[128 * qs:128 * (qs + 1), :],
                                      OUTSB[qs][:, :])

            if loop_iters > 0:
                with tc.For_i(0, loop_iters, 1):
                    body()
            else:
                body()

    nc.compile()
    return nc


def _prep_inputs(x, wq, bq, wk, bk, wv, bv, wo, bo):
    x = np.asarray(x, dtype=np.float32)
    wq = np.asarray(wq, dtype=np.float32)
    wk = np.asarray(wk, dtype=np.float32)
    wv = np.asarray(wv, dtype=np.float32)
    wo = np.asarray(wo, dtype=np.float32)
    for name, b_ in (("bq", bq), ("bk", bk), ("bv", bv)):
        if np.any(np.asarray(b_) != 0):
            raise NotImplementedError(f"nonzero {name} not supported")

    def blockdiag(w):
        o = np.zeros((128, 128), np.float32)
        for i in range(4):
            o[32 * i:32 * (i + 1), 32 * i:32 * (i + 1)] = w
        return o

    # wo rows reordered+zero-padded to match concatT chunk layout:
    # chunk c=2*hg+bank holds head 4*hg+2*bank+? ... head h=(4*hg+jm):
    # c = 2*hg + jm//2, strip = 64*(jm%2); dead strips stay zero.
    wop = np.zeros((16 * 128, D), np.float32)
    for h in range(H):
        hg, jm = h // 4, h % 4
        c = 2 * hg + (jm // 2)
        strip = 64 * (jm % 2)
        wop[128 * c + strip:128 * c + strip + 32, :] = wo[32 * h:32 * (h + 1), :]

    bfl = ml_dtypes.bfloat16
    shared = {
        "wqbd": blockdiag(wq).astype(bfl), "wkbd": blockdiag(wk).astype(bfl),
        "wvbd": blockdiag(wv).astype(bfl), "wop": wop.astype(bfl),
    }
    xts = [np.ascontiguousarray(x[b].T) for b in range(B)]
    in_maps = []
    for c in range(NCORES):
        b, qc = c // (NCORES // B), c % (NCORES // B)
        m = dict(shared)
        m["xt"] = xts[b]
        m["xq"] = np.ascontiguousarray(xts[b][:, QCH * qc:QCH * (qc + 1)])
        in_maps.append(m)
    return in_maps


_NC_CACHE = {}


def kernel(x, wq, bq, wk, bk, wv, bv, wo, bo):
    in_maps = _prep_inputs(x, wq, bq, wk, bk, wv, bv, wo, bo)
    if "nc" not in _NC_CACHE:
        _NC_CACHE["nc"] = build_module()
    nc = _NC_CACHE["nc"]
    res = bass_utils.run_bass_kernel_spmd(nc, in_maps,
                                          core_ids=list(range(NCORES)))
    out = np.empty((B, S, D), np.float32)
    for c in range(NCORES):
        b, qc = c // (NCORES // B), c % (NCORES // B)
        out[b, QCH * qc:QCH * (qc + 1), :] = res.results[c]["out"]
    out += np.asarray(bo, dtype=np.float32)[None, None, :]
    return out
